# revision 9
# baseline (speedup 1.0000x reference)
"""Trainium2 Bass kernel for nn_DistanceModule.

Computes, for h [4,512,64], W [64,64], b/gamma/beta [64]:
    x = LayerNorm(ReLU(h @ W.T + b))          # [B,N,C]
    D[b,i,j,c] = x[b,i,c] * x[b,j,c]
    out = softmax(D, axis=-1)                 # [B,N,N,C] f32 (256 MB)

Sharding: 2048 (b,i) rows split across 8 cores -> 256 rows/core
(core k: batch b=k//2, i in [256*(k%2), 256*(k%2)+256)). Each core
computes x[b] on-chip, then streams its [256, 512, 64] output slice.

Per-core pipeline (all engines overlapped, per (i-tile, j-block) chunk):
  PE    : selector-matmul broadcasts xT row c across 128 partitions (PSUM)
  ScalarE: exp(bcast_c * x_i[:,c]) fused multiply+exp, per c
  VectorE: segmented reduce_sum over c, reciprocal, normalize multiply
  DMA   : 8 MB contiguous store per chunk

Softmax is computed without max-subtraction: LayerNorm bounds |x| by
sqrt(C-1) ~= 7.94, so logits <= 63 and exp <= 2.4e27 < f32 max.
"""

import numpy as np

import concourse.bacc as bacc
import concourse.bass as bass
import concourse.mybir as mybir
import concourse.tile as tile
from concourse.bass_utils import run_bass_kernel_spmd

B, N, C = 4, 512, 64
NCORES = 8
ROWS = 256          # (b,i) rows per core
JBLK = 256          # j-block width
EPS = 1e-5
F32 = mybir.dt.float32
BF16 = mybir.dt.bfloat16

_CACHE = {}


def _build_program():
    nc = bacc.Bacc(
        "TRN2",
        target_bir_lowering=False,
        debug=False,
        enable_asserts=False,
        num_devices=NCORES,
    )

    hT_d = nc.dram_tensor("hT", [C, N], F32, kind="ExternalInput")
    hTi_d = nc.dram_tensor("hTi", [C, ROWS], F32, kind="ExternalInput")
    WT_d = nc.dram_tensor("WT", [C, C], F32, kind="ExternalInput")
    bgb_d = nc.dram_tensor("bgb", [128, 3 * C], F32, kind="ExternalInput")
    sel_d = nc.dram_tensor("sel", [C, C * 128], BF16, kind="ExternalInput")
    id_d = nc.dram_tensor("identity", [128, 128], F32, kind="ExternalInput")
    out_d = nc.dram_tensor("out", [ROWS, N * C], F32, kind="ExternalOutput")

    X = mybir.AxisListType.X
    sub = mybir.AluOpType.subtract
    mult = mybir.AluOpType.mult
    Exp = mybir.ActivationFunctionType.Exp
    Sqrt = mybir.ActivationFunctionType.Sqrt

    with tile.TileContext(nc) as tc:
        with tc.tile_pool(name="const", bufs=1) as constp:
            hT = constp.tile([C, N], F32)
            nc.sync.dma_start(hT[:], hT_d[:])
            hTi = constp.tile([C, ROWS], F32)
            nc.sync.dma_start(hTi[:], hTi_d[:])
            WT = constp.tile([C, C], F32)
            nc.sync.dma_start(WT[:], WT_d[:])
            bgb = constp.tile([128, 3 * C], F32)
            nc.sync.dma_start(bgb[:], bgb_d[:])
            sel = constp.tile([C, C * 128], BF16)
            nc.sync.dma_start(sel[:], sel_d[:])
            ident = constp.tile([128, 128], F32)
            nc.sync.dma_start(ident[:], id_d[:])

            xT = constp.tile([C, N], F32)          # x[b].T  (c on partitions)
            xi = constp.tile([128, 2, C], F32)     # this core's two i-tiles
            eps_t = constp.tile([128, 1], F32)
            nc.vector.memset(eps_t[:], EPS)

            # ---- x = LayerNorm(ReLU(h @ W.T + b)) --------------------------
            with (
                tc.tile_pool(name="xprep", bufs=2) as xprep,
                tc.tile_pool(name="psum_prep", bufs=2, space=bass.MemorySpace.PSUM) as psp,
                tc.tile_pool(name="psum_tp", bufs=2, space=bass.MemorySpace.PSUM) as ptp,
            ):
                for t in range(6):
                    if t < 4:
                        lhsT = hT[:, t * 128:(t + 1) * 128]
                    else:
                        lhsT = hTi[:, (t - 4) * 128:(t - 3) * 128]
                    xp = psp.tile([128, C], F32, tag="xp")
                    nc.tensor.matmul(xp[:], lhsT, WT[:])
                    xs = xprep.tile([128, C], F32, tag="xs")
                    nc.vector.tensor_add(xs[:], xp[:], bgb[:, 0:C])       # + b
                    nc.vector.tensor_scalar_max(xs[:], xs[:], 0.0)        # ReLU
                    stats = xprep.tile([128, 6], F32, tag="stats")
                    nc.vector.bn_stats(stats[:], xs[:])
                    mv = xprep.tile([128, 2], F32, tag="mv")
                    nc.vector.bn_aggr(mv[:], stats[:])
                    std = xprep.tile([128, 1], F32, tag="std")
                    nc.scalar.activation(std[:], mv[:, 1:2], Sqrt, bias=eps_t[:, 0:1])
                    rstd = xprep.tile([128, 1], F32, tag="rstd")
                    nc.vector.reciprocal(rstd[:], std[:])
                    xn = xprep.tile([128, C], F32, tag="xn")
                    nc.vector.tensor_scalar(
                        xn[:], xs[:], mv[:, 0:1], rstd[:, 0:1], op0=sub, op1=mult
                    )
                    nc.vector.tensor_mul(xn[:], xn[:], bgb[:, C:2 * C])   # * gamma
                    nc.vector.tensor_add(xn[:], xn[:], bgb[:, 2 * C:3 * C])  # + beta
                    if t < 4:
                        tp = ptp.tile([C, 128], F32, tag="tp")
                        nc.tensor.transpose(tp[:], xn[:], ident[:])
                        nc.vector.tensor_copy(xT[:, t * 128:(t + 1) * 128], tp[:])
                    else:
                        nc.vector.tensor_copy(xi[:, t - 4, :], xn[:])

            # hi/lo bf16 split of xT: x = hi + lo exactly to ~2^-17, so the
            # bf16 matmul pair (PSUM accumulates in fp32) reproduces the f32
            # broadcast at ~4x the fp32 matmul speed.
            xT_hi = constp.tile([C, N], BF16)
            nc.vector.tensor_copy(xT_hi[:], xT[:])
            hi32 = constp.tile([C, N], F32)
            nc.vector.tensor_copy(hi32[:], xT_hi[:])
            xT_lo = constp.tile([C, N], BF16)
            nc.vector.tensor_sub(xT_lo[:], xT[:], hi32[:])

            # ---- main: exp(x_i * x_j), softmax over c, store ---------------
            # exp tiles are bf16 (output rounding ~2^-9): halves SBUF so a
            # full-j (FD=512) activation fits double-buffered, halving the
            # ScalarE instruction count. The output DMA casts bf16->f32.
            NQ = 4
            QW = N // NQ
            with (
                tc.tile_pool(name="main", bufs=2) as mainp,
                tc.tile_pool(name="small", bufs=4) as smallp,
                tc.tile_pool(name="psum_bc", bufs=4, space=bass.MemorySpace.PSUM) as pbc,
            ):
                for it in range(2):
                    expt = mainp.tile([128, N, C], BF16, tag="exp")
                    for c in range(C):
                        bc = pbc.tile([128, N], F32, tag="bc")
                        nc.tensor.matmul(
                            bc[:], sel[:, c * 128:(c + 1) * 128], xT_hi[:],
                            start=True, stop=False,
                        )
                        nc.tensor.matmul(
                            bc[:], sel[:, c * 128:(c + 1) * 128], xT_lo[:],
                            start=False, stop=True,
                        )
                        nc.scalar.activation(
                            expt[:, :, c], bc[:], Exp, scale=xi[:, it, c:c + 1]
                        )
                    for q in range(NQ):
                        sl = slice(q * QW, (q + 1) * QW)
                        sums = smallp.tile([128, QW], F32, tag="sums")
                        nc.vector.reduce_sum(sums[:], expt[:, sl, :], axis=X)
                        recip = smallp.tile([128, QW], F32, tag="recip")
                        nc.vector.reciprocal(recip[:], sums[:])
                        nc.vector.tensor_mul(
                            expt[:, sl, :],
                            expt[:, sl, :],
                            recip[:, :, None].broadcast_to((128, QW, C)),
                        )
                        nc.gpsimd.dma_start(
                            out_d[it * 128:(it + 1) * 128,
                                  q * QW * C:(q + 1) * QW * C],
                            expt[:, sl, :].rearrange("p j c -> p (j c)"),
                        )
    nc.compile()
    return nc


def _in_maps(h, W, b, gamma, beta):
    h = np.asarray(h, dtype=np.float32)
    W = np.asarray(W, dtype=np.float32)
    b = np.asarray(b, dtype=np.float32)
    gamma = np.asarray(gamma, dtype=np.float32)
    beta = np.asarray(beta, dtype=np.float32)

    WT = np.ascontiguousarray(W.T)
    bgb = np.ascontiguousarray(
        np.broadcast_to(np.concatenate([b, gamma, beta])[None, :], (128, 3 * C))
    )
    import ml_dtypes
    sel = np.zeros((C, C * 128), dtype=ml_dtypes.bfloat16)
    for c in range(C):
        sel[c, c * 128:(c + 1) * 128] = 1.0
    ident = np.eye(128, dtype=np.float32)

    in_maps = []
    for k in range(NCORES):
        bb, half = divmod(k, 2)
        i0 = half * ROWS
        in_maps.append({
            "hT": np.ascontiguousarray(h[bb].T),
            "hTi": np.ascontiguousarray(h[bb, i0:i0 + ROWS].T),
            "WT": WT,
            "bgb": bgb,
            "sel": sel,
            "identity": ident,
        })
    return in_maps


def run(h, W, b, gamma, beta, trace=False, **trace_kwargs):
    if "nc" not in _CACHE:
        _CACHE["nc"] = _build_program()
    nc = _CACHE["nc"]
    res = run_bass_kernel_spmd(
        nc,
        _in_maps(h, W, b, gamma, beta),
        core_ids=list(range(NCORES)),
        trace=trace,
        **trace_kwargs,
    )
    out = np.zeros((B, N, N, C), dtype=np.float32)
    for k in range(NCORES):
        bb, half = divmod(k, 2)
        i0 = half * ROWS
        out[bb, i0:i0 + ROWS] = res.results[k]["out"].reshape(ROWS, N, C)
    return out, res


def kernel(h, W, b, gamma, beta):
    out, _ = run(h, W, b, gamma, beta)
    return out


# revision 12
# speedup vs baseline: 1.2877x; 1.2877x over previous
"""Trainium2 Bass kernel for nn_DistanceModule.

Computes, for h [4,512,64], W [64,64], b/gamma/beta [64]:
    x = LayerNorm(ReLU(h @ W.T + b))          # [B,N,C]
    D[b,i,j,c] = x[b,i,c] * x[b,j,c]
    out = softmax(D, axis=-1)                 # [B,N,N,C] f32 (256 MB)

Sharding: 2048 (b,i) rows split across 8 cores -> 256 rows/core
(core k: batch b=k//2, i in [256*(k%2), 256*(k%2)+256)). Each core
computes x[b] on-chip, then streams its [256, 512, 64] output slice.

Per-core pipeline (all engines overlapped, per (i-tile, j-block) chunk):
  PE    : selector-matmul broadcasts xT row c across 128 partitions (PSUM)
  ScalarE: exp(bcast_c * x_i[:,c]) fused multiply+exp, per c
  VectorE: segmented reduce_sum over c, reciprocal, normalize multiply
  DMA   : 8 MB contiguous store per chunk

Softmax is computed without max-subtraction: LayerNorm bounds |x| by
sqrt(C-1) ~= 7.94, so logits <= 63 and exp <= 2.4e27 < f32 max.
"""

import numpy as np

import concourse.bacc as bacc
import concourse.bass as bass
import concourse.mybir as mybir
import concourse.tile as tile
from concourse.bass_utils import run_bass_kernel_spmd

B, N, C = 4, 512, 64
NCORES = 8
ROWS = 256          # (b,i) rows per core
JBLK = 256          # j-block width
EPS = 1e-5
F32 = mybir.dt.float32
BF16 = mybir.dt.bfloat16

_CACHE = {}


def _build_program():
    nc = bacc.Bacc(
        "TRN2",
        target_bir_lowering=False,
        debug=False,
        enable_asserts=False,
        num_devices=NCORES,
    )

    hT_d = nc.dram_tensor("hT", [C, N], F32, kind="ExternalInput")
    hTi_d = nc.dram_tensor("hTi", [C, ROWS], F32, kind="ExternalInput")
    WT_d = nc.dram_tensor("WT", [C, C], F32, kind="ExternalInput")
    bgb_d = nc.dram_tensor("bgb", [128, 3 * C], F32, kind="ExternalInput")
    sel_d = nc.dram_tensor("sel", [C, C * 128], BF16, kind="ExternalInput")
    id_d = nc.dram_tensor("identity", [128, 128], F32, kind="ExternalInput")
    out_d = nc.dram_tensor("out", [ROWS, N * C], F32, kind="ExternalOutput")

    X = mybir.AxisListType.X
    sub = mybir.AluOpType.subtract
    mult = mybir.AluOpType.mult
    Exp = mybir.ActivationFunctionType.Exp
    Sqrt = mybir.ActivationFunctionType.Sqrt

    with tile.TileContext(nc) as tc:
        with tc.tile_pool(name="const", bufs=1) as constp:
            sel = constp.tile([C, C * 128], BF16)
            nc.sync.dma_start(sel[:], sel_d[:])

            xT = constp.tile([C, N], F32)          # x[b].T  (c on partitions)
            xi = constp.tile([128, 2, C], F32)     # this core's two i-tiles
            xT_hi = constp.tile([C, N], BF16)
            xT_lo = constp.tile([C, N], BF16)

            # ---- x = LayerNorm(ReLU(h @ W.T + b)) --------------------------
            with (
                tc.tile_pool(name="prepc", bufs=1) as prepc,
                tc.tile_pool(name="xprep", bufs=2) as xprep,
                tc.tile_pool(name="psum_prep", bufs=2, space=bass.MemorySpace.PSUM) as psp,
                tc.tile_pool(name="psum_tp", bufs=2, space=bass.MemorySpace.PSUM) as ptp,
            ):
                hT = prepc.tile([C, N], F32)
                nc.sync.dma_start(hT[:], hT_d[:])
                hTi = prepc.tile([C, ROWS], F32)
                nc.sync.dma_start(hTi[:], hTi_d[:])
                WT = prepc.tile([C, C], F32)
                nc.sync.dma_start(WT[:], WT_d[:])
                bgb = prepc.tile([128, 3 * C], F32)
                nc.sync.dma_start(bgb[:], bgb_d[:])
                ident = prepc.tile([128, 128], F32)
                nc.sync.dma_start(ident[:], id_d[:])
                eps_t = prepc.tile([128, 1], F32)
                nc.vector.memset(eps_t[:], EPS)
                for t in range(6):
                    if t < 4:
                        lhsT = hT[:, t * 128:(t + 1) * 128]
                    else:
                        lhsT = hTi[:, (t - 4) * 128:(t - 3) * 128]
                    xp = psp.tile([128, C], F32, tag="xp")
                    nc.tensor.matmul(xp[:], lhsT, WT[:])
                    xs = xprep.tile([128, C], F32, tag="xs")
                    nc.vector.tensor_add(xs[:], xp[:], bgb[:, 0:C])       # + b
                    nc.vector.tensor_scalar_max(xs[:], xs[:], 0.0)        # ReLU
                    stats = xprep.tile([128, 6], F32, tag="stats")
                    nc.vector.bn_stats(stats[:], xs[:])
                    mv = xprep.tile([128, 2], F32, tag="mv")
                    nc.vector.bn_aggr(mv[:], stats[:])
                    std = xprep.tile([128, 1], F32, tag="std")
                    nc.scalar.activation(std[:], mv[:, 1:2], Sqrt, bias=eps_t[:, 0:1])
                    rstd = xprep.tile([128, 1], F32, tag="rstd")
                    nc.vector.reciprocal(rstd[:], std[:])
                    xn = xprep.tile([128, C], F32, tag="xn")
                    nc.vector.tensor_scalar(
                        xn[:], xs[:], mv[:, 0:1], rstd[:, 0:1], op0=sub, op1=mult
                    )
                    nc.vector.tensor_mul(xn[:], xn[:], bgb[:, C:2 * C])   # * gamma
                    nc.vector.tensor_add(xn[:], xn[:], bgb[:, 2 * C:3 * C])  # + beta
                    if t < 4:
                        tp = ptp.tile([C, 128], F32, tag="tp")
                        nc.tensor.transpose(tp[:], xn[:], ident[:])
                        nc.vector.tensor_copy(xT[:, t * 128:(t + 1) * 128], tp[:])
                    else:
                        nc.vector.tensor_copy(xi[:, t - 4, :], xn[:])

                # hi/lo bf16 split of xT: x = hi + lo exactly to ~2^-17, so
                # the bf16 matmul pair (PSUM accumulates in fp32) reproduces
                # the f32 broadcast at ~4x the fp32 matmul speed.
                nc.vector.tensor_copy(xT_hi[:], xT[:])
                hi32 = prepc.tile([C, N], F32)
                nc.vector.tensor_copy(hi32[:], xT_hi[:])
                nc.vector.tensor_sub(xT_lo[:], xT[:], hi32[:])

            # ---- main: exp(x_i * x_j), softmax over c, store ---------------
            # exp tiles are bf16 stored c-major [128, c, j]: each FD=512
            # activation writes a contiguous [128, 512] row (no strided-bf16
            # RMW), and halving the tile size lets two i-tiles pipeline.
            # Normalize runs in f32 into small j-major staging tiles which
            # are DMA'd out as plain f32.
            NQ = 8
            QW = N // NQ  # 64 j per staging tile
            with (
                tc.tile_pool(name="main", bufs=2) as mainp,
                tc.tile_pool(name="stage", bufs=3) as stagep,
                tc.tile_pool(name="small", bufs=2) as smallp,
                tc.tile_pool(name="psum_bc", bufs=4, space=bass.MemorySpace.PSUM) as pbc,
            ):
                for it in range(2):
                    expt = mainp.tile([128, C, N], BF16, tag="exp")
                    for c in range(C):
                        bc = pbc.tile([128, N], F32, tag="bc")
                        nc.tensor.matmul(
                            bc[:], sel[:, c * 128:(c + 1) * 128], xT_hi[:],
                            start=True, stop=False,
                        )
                        nc.tensor.matmul(
                            bc[:], sel[:, c * 128:(c + 1) * 128], xT_lo[:],
                            start=False, stop=True,
                        )
                        nc.scalar.activation(
                            expt[:, c, :], bc[:], Exp, scale=xi[:, it, c:c + 1]
                        )
                    sums = smallp.tile([128, N], F32, tag="sums")
                    nc.vector.reduce_sum(
                        sums[:], expt[:].rearrange("p c j -> p j c"), axis=X
                    )
                    recip = smallp.tile([128, N], F32, tag="recip")
                    nc.vector.reciprocal(recip[:], sums[:])
                    for q in range(NQ):
                        sl = slice(q * QW, (q + 1) * QW)
                        oute = stagep.tile([128, QW, C], F32, tag="oute")
                        nc.vector.tensor_mul(
                            oute[:],
                            expt[:, :, sl].rearrange("p c j -> p j c"),
                            recip[:, sl][:, :, None].broadcast_to((128, QW, C)),
                        )
                        nc.sync.dma_start(
                            out_d[it * 128:(it + 1) * 128,
                                  q * QW * C:(q + 1) * QW * C],
                            oute[:].rearrange("p j c -> p (j c)"),
                        )
    nc.compile()
    return nc


def _in_maps(h, W, b, gamma, beta):
    h = np.asarray(h, dtype=np.float32)
    W = np.asarray(W, dtype=np.float32)
    b = np.asarray(b, dtype=np.float32)
    gamma = np.asarray(gamma, dtype=np.float32)
    beta = np.asarray(beta, dtype=np.float32)

    WT = np.ascontiguousarray(W.T)
    bgb = np.ascontiguousarray(
        np.broadcast_to(np.concatenate([b, gamma, beta])[None, :], (128, 3 * C))
    )
    import ml_dtypes
    sel = np.zeros((C, C * 128), dtype=ml_dtypes.bfloat16)
    for c in range(C):
        sel[c, c * 128:(c + 1) * 128] = 1.0
    ident = np.eye(128, dtype=np.float32)

    in_maps = []
    for k in range(NCORES):
        bb, half = divmod(k, 2)
        i0 = half * ROWS
        in_maps.append({
            "hT": np.ascontiguousarray(h[bb].T),
            "hTi": np.ascontiguousarray(h[bb, i0:i0 + ROWS].T),
            "WT": WT,
            "bgb": bgb,
            "sel": sel,
            "identity": ident,
        })
    return in_maps


def run(h, W, b, gamma, beta, trace=False, **trace_kwargs):
    if "nc" not in _CACHE:
        _CACHE["nc"] = _build_program()
    nc = _CACHE["nc"]
    res = run_bass_kernel_spmd(
        nc,
        _in_maps(h, W, b, gamma, beta),
        core_ids=list(range(NCORES)),
        trace=trace,
        **trace_kwargs,
    )
    out = np.zeros((B, N, N, C), dtype=np.float32)
    for k in range(NCORES):
        bb, half = divmod(k, 2)
        i0 = half * ROWS
        out[bb, i0:i0 + ROWS] = res.results[k]["out"].reshape(ROWS, N, C)
    return out, res


def kernel(h, W, b, gamma, beta):
    out, _ = run(h, W, b, gamma, beta)
    return out


# revision 18
# speedup vs baseline: 1.5862x; 1.2318x over previous
"""Trainium2 Bass kernel for nn_DistanceModule.

Computes, for h [4,512,64], W [64,64], b/gamma/beta [64]:
    x = LayerNorm(ReLU(h @ W.T + b))          # [B,N,C]
    D[b,i,j,c] = x[b,i,c] * x[b,j,c]
    out = softmax(D, axis=-1)                 # [B,N,N,C] f32 (256 MB)

Sharding: 2048 (b,i) rows split across 8 cores -> 256 rows/core
(core k: batch b=k//2, i in [256*(k%2), 256*(k%2)+256)). Each core
computes x[b] on-chip, then streams its [256, 512, 64] output slice.

Per-core pipeline (all engines overlapped, per (i-tile, j-block) chunk):
  PE    : selector-matmul broadcasts xT row c across 128 partitions (PSUM)
  ScalarE: exp(bcast_c * x_i[:,c]) fused multiply+exp, per c
  VectorE: segmented reduce_sum over c, reciprocal, normalize multiply
  DMA   : 8 MB contiguous store per chunk

Softmax is computed without max-subtraction: LayerNorm bounds |x| by
sqrt(C-1) ~= 7.94, so logits <= 63 and exp <= 2.4e27 < f32 max.
"""

import numpy as np

import concourse.bacc as bacc
import concourse.bass as bass
import concourse.mybir as mybir
import concourse.tile as tile
from concourse.bass_utils import run_bass_kernel_spmd

B, N, C = 4, 512, 64
NCORES = 8
ROWS = 256          # (b,i) rows per core
JBLK = 256          # j-block width
EPS = 1e-5
F32 = mybir.dt.float32
BF16 = mybir.dt.bfloat16

_CACHE = {}


def _build_program():
    nc = bacc.Bacc(
        "TRN2",
        target_bir_lowering=False,
        debug=False,
        enable_asserts=False,
        num_devices=NCORES,
    )

    hT_d = nc.dram_tensor("hT", [C, N], F32, kind="ExternalInput")
    hTi_d = nc.dram_tensor("hTi", [C, ROWS], F32, kind="ExternalInput")
    WT_d = nc.dram_tensor("WT", [C, C], F32, kind="ExternalInput")
    bgb_d = nc.dram_tensor("bgb", [128, 3 * C], F32, kind="ExternalInput")
    sel_d = nc.dram_tensor("sel", [2 * C, C * 128], BF16, kind="ExternalInput")
    id_d = nc.dram_tensor("identity", [128, 128], F32, kind="ExternalInput")
    out_d = nc.dram_tensor("out", [ROWS, N * C], F32, kind="ExternalOutput")

    X = mybir.AxisListType.X
    sub = mybir.AluOpType.subtract
    mult = mybir.AluOpType.mult
    Exp = mybir.ActivationFunctionType.Exp
    Sqrt = mybir.ActivationFunctionType.Sqrt

    with tile.TileContext(nc) as tc:
        with tc.tile_pool(name="const", bufs=1) as constp:
            hT = constp.tile([C, N], F32)
            nc.sync.dma_start(hT[:], hT_d[:])
            hTi = constp.tile([C, ROWS], F32)
            nc.sync.dma_start(hTi[:], hTi_d[:])
            WT = constp.tile([C, C], F32)
            nc.sync.dma_start(WT[:], WT_d[:])
            bgb = constp.tile([128, 3 * C], F32)
            nc.sync.dma_start(bgb[:], bgb_d[:])
            sel = constp.tile([2 * C, C * 128], BF16)
            nc.sync.dma_start(sel[:], sel_d[:])
            ident = constp.tile([128, 128], F32)
            nc.sync.dma_start(ident[:], id_d[:])

            xT = constp.tile([C, N], F32)          # x[b].T  (c on partitions)
            xi = constp.tile([128, 2, C], F32)     # this core's two i-tiles
            eps_t = constp.tile([128, 1], F32)
            nc.vector.memset(eps_t[:], EPS)

            # ---- x = LayerNorm(ReLU(h @ W.T + b)) --------------------------
            with (
                tc.tile_pool(name="xprep", bufs=2) as xprep,
                tc.tile_pool(name="psum_prep", bufs=2, space=bass.MemorySpace.PSUM) as psp,
                tc.tile_pool(name="psum_tp", bufs=2, space=bass.MemorySpace.PSUM) as ptp,
            ):
                for t in range(6):
                    if t < 4:
                        lhsT = hT[:, t * 128:(t + 1) * 128]
                    else:
                        lhsT = hTi[:, (t - 4) * 128:(t - 3) * 128]
                    xp = psp.tile([128, C], F32, tag="xp")
                    nc.tensor.matmul(xp[:], lhsT, WT[:])
                    xs = xprep.tile([128, C], F32, tag="xs")
                    nc.vector.tensor_add(xs[:], xp[:], bgb[:, 0:C])       # + b
                    nc.vector.tensor_scalar_max(xs[:], xs[:], 0.0)        # ReLU
                    stats = xprep.tile([128, 6], F32, tag="stats")
                    nc.vector.bn_stats(stats[:], xs[:])
                    mv = xprep.tile([128, 2], F32, tag="mv")
                    nc.vector.bn_aggr(mv[:], stats[:])
                    std = xprep.tile([128, 1], F32, tag="std")
                    nc.scalar.activation(std[:], mv[:, 1:2], Sqrt, bias=eps_t[:, 0:1])
                    rstd = xprep.tile([128, 1], F32, tag="rstd")
                    nc.vector.reciprocal(rstd[:], std[:])
                    xn = xprep.tile([128, C], F32, tag="xn")
                    nc.vector.tensor_scalar(
                        xn[:], xs[:], mv[:, 0:1], rstd[:, 0:1], op0=sub, op1=mult
                    )
                    nc.vector.tensor_mul(xn[:], xn[:], bgb[:, C:2 * C])   # * gamma
                    nc.vector.tensor_add(xn[:], xn[:], bgb[:, 2 * C:3 * C])  # + beta
                    if t < 4:
                        tp = ptp.tile([C, 128], F32, tag="tp")
                        nc.tensor.transpose(tp[:], xn[:], ident[:])
                        nc.vector.tensor_copy(xT[:, t * 128:(t + 1) * 128], tp[:])
                    else:
                        nc.vector.tensor_copy(xi[:, t - 4, :], xn[:])

            # hi/lo bf16 split of xT stacked on the K axis: partitions 0-63
            # hold bf16(x), 64-127 hold bf16(x - hi). One K=128 matmul with
            # the doubled selector sums both rank-64 halves in PSUM fp32,
            # reproducing the f32 broadcast exactly to ~2^-17 in a single
            # pass (no same-bank accumulate serialization).
            xT_hilo = constp.tile([128, N], BF16)
            nc.vector.tensor_copy(xT_hilo[0:C, :], xT[:])
            hi32 = constp.tile([C, N], F32)
            nc.vector.tensor_copy(hi32[:], xT_hilo[0:C, :])
            nc.vector.tensor_sub(xT_hilo[C:2 * C, :], xT[:], hi32[:])

            # ---- main: exp(x_i * x_j), softmax over c, store ---------------
            with (
                tc.tile_pool(name="main", bufs=2) as mainp,
                tc.tile_pool(name="small", bufs=3) as smallp,
                tc.tile_pool(name="psum_bc", bufs=4, space=bass.MemorySpace.PSUM) as pbc,
            ):
                for it in range(2):
                    for jb in range(N // JBLK):
                        expt = mainp.tile([128, JBLK, C], F32, tag="exp")
                        for c in range(C):
                            bc = pbc.tile([128, JBLK], F32, tag="bc")
                            nc.tensor.matmul(
                                bc[:],
                                sel[:, c * 128:(c + 1) * 128],
                                xT_hilo[:, jb * JBLK:(jb + 1) * JBLK],
                            )
                            nc.scalar.activation(
                                expt[:, :, c], bc[:], Exp, scale=xi[:, it, c:c + 1]
                            )
                        sums = smallp.tile([128, JBLK], F32, tag="sums")
                        nc.vector.reduce_sum(sums[:], expt[:], axis=X)
                        recip = smallp.tile([128, JBLK], F32, tag="recip")
                        nc.vector.reciprocal(recip[:], sums[:])
                        nc.vector.tensor_mul(
                            expt[:],
                            expt[:],
                            recip[:, :, None].broadcast_to((128, JBLK, C)),
                        )
                        nc.sync.dma_start(
                            out_d[it * 128:(it + 1) * 128,
                                  jb * JBLK * C:(jb + 1) * JBLK * C],
                            expt[:].rearrange("p j c -> p (j c)"),
                        )
    nc.compile()
    return nc


def _in_maps(h, W, b, gamma, beta):
    h = np.asarray(h, dtype=np.float32)
    W = np.asarray(W, dtype=np.float32)
    b = np.asarray(b, dtype=np.float32)
    gamma = np.asarray(gamma, dtype=np.float32)
    beta = np.asarray(beta, dtype=np.float32)

    WT = np.ascontiguousarray(W.T)
    bgb = np.ascontiguousarray(
        np.broadcast_to(np.concatenate([b, gamma, beta])[None, :], (128, 3 * C))
    )
    import ml_dtypes
    sel = np.zeros((2 * C, C * 128), dtype=ml_dtypes.bfloat16)
    for c in range(C):
        sel[c, c * 128:(c + 1) * 128] = 1.0
        sel[C + c, c * 128:(c + 1) * 128] = 1.0
    ident = np.eye(128, dtype=np.float32)

    in_maps = []
    for k in range(NCORES):
        bb, half = divmod(k, 2)
        i0 = half * ROWS
        in_maps.append({
            "hT": np.ascontiguousarray(h[bb].T),
            "hTi": np.ascontiguousarray(h[bb, i0:i0 + ROWS].T),
            "WT": WT,
            "bgb": bgb,
            "sel": sel,
            "identity": ident,
        })
    return in_maps


def run(h, W, b, gamma, beta, trace=False, **trace_kwargs):
    if "nc" not in _CACHE:
        _CACHE["nc"] = _build_program()
    nc = _CACHE["nc"]
    res = run_bass_kernel_spmd(
        nc,
        _in_maps(h, W, b, gamma, beta),
        core_ids=list(range(NCORES)),
        trace=trace,
        **trace_kwargs,
    )
    out = np.zeros((B, N, N, C), dtype=np.float32)
    for k in range(NCORES):
        bb, half = divmod(k, 2)
        i0 = half * ROWS
        out[bb, i0:i0 + ROWS] = res.results[k]["out"].reshape(ROWS, N, C)
    return out, res


def kernel(h, W, b, gamma, beta):
    out, _ = run(h, W, b, gamma, beta)
    return out


# revision 19
# speedup vs baseline: 1.6235x; 1.0235x over previous
"""Trainium2 Bass kernel for nn_DistanceModule.

Computes, for h [4,512,64], W [64,64], b/gamma/beta [64]:
    x = LayerNorm(ReLU(h @ W.T + b))          # [B,N,C]
    D[b,i,j,c] = x[b,i,c] * x[b,j,c]
    out = softmax(D, axis=-1)                 # [B,N,N,C] f32 (256 MB)

Sharding: 2048 (b,i) rows split across 8 cores -> 256 rows/core
(core k: batch b=k//2, i in [256*(k%2), 256*(k%2)+256)). Each core
computes x[b] on-chip, then streams its [256, 512, 64] output slice.

Per-core pipeline (all engines overlapped, per (i-tile, j-block) chunk):
  PE    : selector-matmul broadcasts xT row c across 128 partitions (PSUM)
  ScalarE: exp(bcast_c * x_i[:,c]) fused multiply+exp, per c
  VectorE: segmented reduce_sum over c, reciprocal, normalize multiply
  DMA   : 8 MB contiguous store per chunk

Softmax is computed without max-subtraction: LayerNorm bounds |x| by
sqrt(C-1) ~= 7.94, so logits <= 63 and exp <= 2.4e27 < f32 max.
"""

import numpy as np

import concourse.bacc as bacc
import concourse.bass as bass
import concourse.mybir as mybir
import concourse.tile as tile
from concourse.bass_utils import run_bass_kernel_spmd

B, N, C = 4, 512, 64
NCORES = 8
ROWS = 256          # (b,i) rows per core
JBLK = 256          # j-block width
EPS = 1e-5
F32 = mybir.dt.float32
BF16 = mybir.dt.bfloat16

_CACHE = {}


def _build_program():
    nc = bacc.Bacc(
        "TRN2",
        target_bir_lowering=False,
        debug=False,
        enable_asserts=False,
        num_devices=NCORES,
    )

    hT_d = nc.dram_tensor("hT", [C, N], F32, kind="ExternalInput")
    hTi_d = nc.dram_tensor("hTi", [C, ROWS], F32, kind="ExternalInput")
    WT_d = nc.dram_tensor("WT", [C, C], F32, kind="ExternalInput")
    bgb_d = nc.dram_tensor("bgb", [128, 3 * C], F32, kind="ExternalInput")
    sel_d = nc.dram_tensor("sel", [2 * C, C * 128], BF16, kind="ExternalInput")
    id_d = nc.dram_tensor("identity", [128, 128], F32, kind="ExternalInput")
    out_d = nc.dram_tensor("out", [ROWS, N * C], F32, kind="ExternalOutput")

    X = mybir.AxisListType.X
    sub = mybir.AluOpType.subtract
    mult = mybir.AluOpType.mult
    Exp = mybir.ActivationFunctionType.Exp
    Sqrt = mybir.ActivationFunctionType.Sqrt

    with tile.TileContext(nc) as tc:
        with tc.tile_pool(name="const", bufs=1) as constp:
            hT = constp.tile([C, N], F32)
            nc.sync.dma_start(hT[:], hT_d[:])
            hTi = constp.tile([C, ROWS], F32)
            nc.sync.dma_start(hTi[:], hTi_d[:])
            WT = constp.tile([C, C], F32)
            nc.sync.dma_start(WT[:], WT_d[:])
            bgb = constp.tile([128, 3 * C], F32)
            nc.sync.dma_start(bgb[:], bgb_d[:])
            sel = constp.tile([2 * C, C * 128], BF16)
            nc.sync.dma_start(sel[:], sel_d[:])
            ident = constp.tile([128, 128], F32)
            nc.sync.dma_start(ident[:], id_d[:])

            xT = constp.tile([C, N], F32)          # x[b].T  (c on partitions)
            xi = constp.tile([128, 2, C], F32)     # this core's two i-tiles
            eps_t = constp.tile([128, 1], F32)
            nc.vector.memset(eps_t[:], EPS)

            # ---- x = LayerNorm(ReLU(h @ W.T + b)) --------------------------
            with (
                tc.tile_pool(name="xprep", bufs=2) as xprep,
                tc.tile_pool(name="psum_prep", bufs=2, space=bass.MemorySpace.PSUM) as psp,
                tc.tile_pool(name="psum_tp", bufs=2, space=bass.MemorySpace.PSUM) as ptp,
            ):
                for t in range(6):
                    if t < 4:
                        lhsT = hT[:, t * 128:(t + 1) * 128]
                    else:
                        lhsT = hTi[:, (t - 4) * 128:(t - 3) * 128]
                    xp = psp.tile([128, C], F32, tag="xp")
                    nc.tensor.matmul(xp[:], lhsT, WT[:])
                    xs = xprep.tile([128, C], F32, tag="xs")
                    nc.vector.tensor_add(xs[:], xp[:], bgb[:, 0:C])       # + b
                    nc.vector.tensor_scalar_max(xs[:], xs[:], 0.0)        # ReLU
                    stats = xprep.tile([128, 6], F32, tag="stats")
                    nc.vector.bn_stats(stats[:], xs[:])
                    mv = xprep.tile([128, 2], F32, tag="mv")
                    nc.vector.bn_aggr(mv[:], stats[:])
                    std = xprep.tile([128, 1], F32, tag="std")
                    nc.scalar.activation(std[:], mv[:, 1:2], Sqrt, bias=eps_t[:, 0:1])
                    rstd = xprep.tile([128, 1], F32, tag="rstd")
                    nc.vector.reciprocal(rstd[:], std[:])
                    xn = xprep.tile([128, C], F32, tag="xn")
                    nc.vector.tensor_scalar(
                        xn[:], xs[:], mv[:, 0:1], rstd[:, 0:1], op0=sub, op1=mult
                    )
                    nc.vector.tensor_mul(xn[:], xn[:], bgb[:, C:2 * C])   # * gamma
                    nc.vector.tensor_add(xn[:], xn[:], bgb[:, 2 * C:3 * C])  # + beta
                    if t < 4:
                        tp = ptp.tile([C, 128], F32, tag="tp")
                        nc.tensor.transpose(tp[:], xn[:], ident[:])
                        nc.vector.tensor_copy(xT[:, t * 128:(t + 1) * 128], tp[:])
                    else:
                        nc.vector.tensor_copy(xi[:, t - 4, :], xn[:])

            # hi/lo bf16 split of xT stacked on the K axis: partitions 0-63
            # hold bf16(x), 64-127 hold bf16(x - hi). One K=128 matmul with
            # the doubled selector sums both rank-64 halves in PSUM fp32,
            # reproducing the f32 broadcast exactly to ~2^-17 in a single
            # pass (no same-bank accumulate serialization).
            xT_hilo = constp.tile([128, N], BF16)
            nc.vector.tensor_copy(xT_hilo[0:C, :], xT[:])
            hi32 = constp.tile([C, N], F32)
            nc.vector.tensor_copy(hi32[:], xT_hilo[0:C, :])
            nc.vector.tensor_sub(xT_hilo[C:2 * C, :], xT[:], hi32[:])

            # ---- main: exp(x_i * x_j), softmax over c, store ---------------
            with (
                tc.tile_pool(name="main", bufs=2) as mainp,
                tc.tile_pool(name="small", bufs=3) as smallp,
                tc.tile_pool(name="psum_bc", bufs=4, space=bass.MemorySpace.PSUM) as pbc,
            ):
                for it in range(2):
                    for jb in range(N // JBLK):
                        expt = mainp.tile([128, JBLK, C], F32, tag="exp")
                        for c in range(C):
                            bc = pbc.tile([128, JBLK], F32, tag="bc")
                            nc.tensor.matmul(
                                bc[:],
                                sel[:, c * 128:(c + 1) * 128],
                                xT_hilo[:, jb * JBLK:(jb + 1) * JBLK],
                            )
                            nc.scalar.activation(
                                expt[:, :, c], bc[:], Exp, scale=xi[:, it, c:c + 1]
                            )
                        sums = smallp.tile([128, JBLK], F32, tag="sums")
                        nc.vector.reduce_sum(sums[:], expt[:], axis=X)
                        recip = smallp.tile([128, JBLK], F32, tag="recip")
                        nc.vector.reciprocal(recip[:], sums[:])
                        # normalize in j-quarters; each quarter DMAs out as
                        # soon as it is scaled (frees the exp buffer sooner
                        # and overlaps store with compute). One quarter runs
                        # on the otherwise-idle GpSimd engine.
                        QW = JBLK // 4
                        for q in range(4):
                            sl = slice(q * QW, (q + 1) * QW)
                            eng = nc.gpsimd if q == 3 else nc.vector
                            eng.tensor_mul(
                                expt[:, sl, :],
                                expt[:, sl, :],
                                recip[:, sl][:, :, None].broadcast_to((128, QW, C)),
                            )
                            nc.sync.dma_start(
                                out_d[it * 128:(it + 1) * 128,
                                      (jb * JBLK + q * QW) * C:
                                      (jb * JBLK + (q + 1) * QW) * C],
                                expt[:, sl, :].rearrange("p j c -> p (j c)"),
                            )
    nc.compile()
    return nc


def _in_maps(h, W, b, gamma, beta):
    h = np.asarray(h, dtype=np.float32)
    W = np.asarray(W, dtype=np.float32)
    b = np.asarray(b, dtype=np.float32)
    gamma = np.asarray(gamma, dtype=np.float32)
    beta = np.asarray(beta, dtype=np.float32)

    WT = np.ascontiguousarray(W.T)
    bgb = np.ascontiguousarray(
        np.broadcast_to(np.concatenate([b, gamma, beta])[None, :], (128, 3 * C))
    )
    import ml_dtypes
    sel = np.zeros((2 * C, C * 128), dtype=ml_dtypes.bfloat16)
    for c in range(C):
        sel[c, c * 128:(c + 1) * 128] = 1.0
        sel[C + c, c * 128:(c + 1) * 128] = 1.0
    ident = np.eye(128, dtype=np.float32)

    in_maps = []
    for k in range(NCORES):
        bb, half = divmod(k, 2)
        i0 = half * ROWS
        in_maps.append({
            "hT": np.ascontiguousarray(h[bb].T),
            "hTi": np.ascontiguousarray(h[bb, i0:i0 + ROWS].T),
            "WT": WT,
            "bgb": bgb,
            "sel": sel,
            "identity": ident,
        })
    return in_maps


def run(h, W, b, gamma, beta, trace=False, **trace_kwargs):
    if "nc" not in _CACHE:
        _CACHE["nc"] = _build_program()
    nc = _CACHE["nc"]
    res = run_bass_kernel_spmd(
        nc,
        _in_maps(h, W, b, gamma, beta),
        core_ids=list(range(NCORES)),
        trace=trace,
        **trace_kwargs,
    )
    out = np.zeros((B, N, N, C), dtype=np.float32)
    for k in range(NCORES):
        bb, half = divmod(k, 2)
        i0 = half * ROWS
        out[bb, i0:i0 + ROWS] = res.results[k]["out"].reshape(ROWS, N, C)
    return out, res


def kernel(h, W, b, gamma, beta):
    out, _ = run(h, W, b, gamma, beta)
    return out


# revision 22
# speedup vs baseline: 1.7979x; 1.1074x over previous
"""Trainium2 Bass kernel for nn_DistanceModule.

Computes, for h [4,512,64], W [64,64], b/gamma/beta [64]:
    x = LayerNorm(ReLU(h @ W.T + b))          # [B,N,C]
    D[b,i,j,c] = x[b,i,c] * x[b,j,c]
    out = softmax(D, axis=-1)                 # [B,N,N,C] f32 (256 MB)

Sharding: 2048 (b,i) rows split across 8 cores -> 256 rows/core
(core k: batch b=k//2, i in [256*(k%2), 256*(k%2)+256)). Each core
computes x[b] on-chip, then streams its [256, 512, 64] output slice.

Per-core pipeline (all engines overlapped, per (i-tile, j-block) chunk):
  PE    : selector-matmul broadcasts xT row c across 128 partitions (PSUM)
  ScalarE: exp(bcast_c * x_i[:,c]) fused multiply+exp, per c
  VectorE: segmented reduce_sum over c, reciprocal, normalize multiply
  DMA   : 8 MB contiguous store per chunk

Softmax is computed without max-subtraction: LayerNorm bounds |x| by
sqrt(C-1) ~= 7.94, so logits <= 63 and exp <= 2.4e27 < f32 max.
"""

import numpy as np

import concourse.bacc as bacc
import concourse.bass as bass
import concourse.mybir as mybir
import concourse.tile as tile
from concourse.bass_utils import run_bass_kernel_spmd

B, N, C = 4, 512, 64
NCORES = 8
ROWS = 256          # (b,i) rows per core
JBLK = 256          # j-block width
EPS = 1e-5
F32 = mybir.dt.float32
BF16 = mybir.dt.bfloat16

_CACHE = {}


def _build_program():
    nc = bacc.Bacc(
        "TRN2",
        target_bir_lowering=False,
        debug=False,
        enable_asserts=False,
        num_devices=NCORES,
    )

    hT_d = nc.dram_tensor("hT", [C, N], F32, kind="ExternalInput")
    hTi_d = nc.dram_tensor("hTi", [C, ROWS], F32, kind="ExternalInput")
    WT_d = nc.dram_tensor("WT", [C, C], F32, kind="ExternalInput")
    bgb_d = nc.dram_tensor("bgb", [128, 3 * C], F32, kind="ExternalInput")
    sel_d = nc.dram_tensor("sel", [2 * C, C * 128], BF16, kind="ExternalInput")
    id_d = nc.dram_tensor("identity", [128, 128], F32, kind="ExternalInput")
    out_d = nc.dram_tensor("out", [ROWS, N * C], F32, kind="ExternalOutput")

    X = mybir.AxisListType.X
    sub = mybir.AluOpType.subtract
    mult = mybir.AluOpType.mult
    Exp = mybir.ActivationFunctionType.Exp
    Sqrt = mybir.ActivationFunctionType.Sqrt

    with tile.TileContext(nc) as tc:
        with tc.tile_pool(name="const", bufs=1) as constp:
            hTi = constp.tile([C, ROWS], F32)
            nc.sync.dma_start(hTi[:], hTi_d[:])
            hT = constp.tile([C, N], F32)
            nc.sync.dma_start(hT[:], hT_d[:])
            WT = constp.tile([C, C], F32)
            nc.sync.dma_start(WT[:], WT_d[:])
            bgb = constp.tile([128, 3 * C], F32)
            nc.sync.dma_start(bgb[:], bgb_d[:])
            sel = constp.tile([2 * C, C * 128], BF16)
            nc.sync.dma_start(sel[:], sel_d[:])
            ident = constp.tile([128, 128], F32)
            nc.sync.dma_start(ident[:], id_d[:])

            xT = constp.tile([C, N], F32)          # x[b].T  (c on partitions)
            xi = constp.tile([128, 2, C], F32)     # this core's two i-tiles
            xT_hilo = constp.tile([128, N], BF16)  # K-stacked bf16 hi/lo of xT
            eps_t = constp.tile([128, 1], F32)
            nc.vector.memset(eps_t[:], EPS)

            # ---- x = LayerNorm(ReLU(h @ W.T + b)) --------------------------
            # i-tiles (t=4,5) first so the main loop's scale operand is ready
            # early; each xT slice gets its bf16 hi/lo split as soon as it is
            # transposed, letting the first broadcast matmuls start before
            # the whole prep finishes.
            with (
                tc.tile_pool(name="xprep", bufs=2) as xprep,
                tc.tile_pool(name="psum_prep", bufs=2, space=bass.MemorySpace.PSUM) as psp,
                tc.tile_pool(name="psum_tp", bufs=2, space=bass.MemorySpace.PSUM) as ptp,
            ):
                for t in (4, 5, 0, 1, 2, 3):
                    if t < 4:
                        lhsT = hT[:, t * 128:(t + 1) * 128]
                    else:
                        lhsT = hTi[:, (t - 4) * 128:(t - 3) * 128]
                    xp = psp.tile([128, C], F32, tag="xp")
                    nc.tensor.matmul(xp[:], lhsT, WT[:])
                    xs = xprep.tile([128, C], F32, tag="xs")
                    nc.vector.tensor_add(xs[:], xp[:], bgb[:, 0:C])       # + b
                    nc.vector.tensor_scalar_max(xs[:], xs[:], 0.0)        # ReLU
                    stats = xprep.tile([128, 6], F32, tag="stats")
                    nc.vector.bn_stats(stats[:], xs[:])
                    mv = xprep.tile([128, 2], F32, tag="mv")
                    nc.vector.bn_aggr(mv[:], stats[:])
                    std = xprep.tile([128, 1], F32, tag="std")
                    nc.scalar.activation(std[:], mv[:, 1:2], Sqrt, bias=eps_t[:, 0:1])
                    rstd = xprep.tile([128, 1], F32, tag="rstd")
                    nc.vector.reciprocal(rstd[:], std[:])
                    xn = xprep.tile([128, C], F32, tag="xn")
                    nc.vector.tensor_scalar(
                        xn[:], xs[:], mv[:, 0:1], rstd[:, 0:1], op0=sub, op1=mult
                    )
                    nc.vector.tensor_mul(xn[:], xn[:], bgb[:, C:2 * C])   # * gamma
                    nc.vector.tensor_add(xn[:], xn[:], bgb[:, 2 * C:3 * C])  # + beta
                    if t < 4:
                        tp = ptp.tile([C, 128], F32, tag="tp")
                        nc.tensor.transpose(tp[:], xn[:], ident[:])
                        sl = slice(t * 128, (t + 1) * 128)
                        nc.vector.tensor_copy(xT[:, sl], tp[:])
                        # K-stacked bf16 hi/lo split of this slice:
                        # partitions 0-63 hold bf16(x), 64-127 bf16(x - hi).
                        # One K=128 matmul with the doubled selector then
                        # sums both rank-64 halves in PSUM fp32, reproducing
                        # the f32 broadcast exactly to ~2^-17 in one pass.
                        nc.vector.tensor_copy(xT_hilo[0:C, sl], xT[:, sl])
                        hi32 = xprep.tile([C, 128], F32, tag="hi32")
                        nc.vector.tensor_copy(hi32[:], xT_hilo[0:C, sl])
                        nc.vector.tensor_sub(xT_hilo[C:2 * C, sl], xT[:, sl], hi32[:])
                    else:
                        nc.vector.tensor_copy(xi[:, t - 4, :], xn[:])

            # ---- main: exp(x_i * x_j), softmax over c, store ---------------
            # Chunk widths are asymmetric: a narrow first chunk lets the
            # vector engine (the critical path) start early, and a narrow
            # last chunk shrinks the drain tail. Still 2 activation
            # instructions per (i-tile, c), so ScalarE time is unchanged.
            CHUNKS = {0: (192, 320), 1: (320, 192)}
            with (
                tc.tile_pool(name="main", bufs=2) as mainp,
                tc.tile_pool(name="small", bufs=3) as smallp,
                tc.tile_pool(name="psum_bc", bufs=6, space=bass.MemorySpace.PSUM) as pbc,
            ):
                for it in range(2):
                    j0 = 0
                    for jw in CHUNKS[it]:
                        expt = mainp.tile([128, jw, C], F32, tag="exp")
                        for c in range(C):
                            bc = pbc.tile([128, jw], F32, tag="bc")
                            nc.tensor.matmul(
                                bc[:],
                                sel[:, c * 128:(c + 1) * 128],
                                xT_hilo[:, j0:j0 + jw],
                            )
                            nc.scalar.activation(
                                expt[:, :, c], bc[:], Exp, scale=xi[:, it, c:c + 1]
                            )
                        sums = smallp.tile([128, jw], F32, tag="sums")
                        nc.vector.reduce_sum(sums[:], expt[:], axis=X)
                        recip = smallp.tile([128, jw], F32, tag="recip")
                        nc.vector.reciprocal(recip[:], sums[:])
                        # normalize in j-quarters; each quarter DMAs out as
                        # soon as it is scaled (frees the exp buffer sooner
                        # and overlaps store with compute).
                        QW = jw // 4
                        for q in range(4):
                            sl = slice(q * QW, (q + 1) * QW)
                            nc.vector.tensor_mul(
                                expt[:, sl, :],
                                expt[:, sl, :],
                                recip[:, sl][:, :, None].broadcast_to((128, QW, C)),
                            )
                            nc.sync.dma_start(
                                out_d[it * 128:(it + 1) * 128,
                                      (j0 + q * QW) * C:(j0 + (q + 1) * QW) * C],
                                expt[:, sl, :].rearrange("p j c -> p (j c)"),
                            )
                        j0 += jw
    nc.compile()
    return nc


def _in_maps(h, W, b, gamma, beta):
    h = np.asarray(h, dtype=np.float32)
    W = np.asarray(W, dtype=np.float32)
    b = np.asarray(b, dtype=np.float32)
    gamma = np.asarray(gamma, dtype=np.float32)
    beta = np.asarray(beta, dtype=np.float32)

    WT = np.ascontiguousarray(W.T)
    bgb = np.ascontiguousarray(
        np.broadcast_to(np.concatenate([b, gamma, beta])[None, :], (128, 3 * C))
    )
    import ml_dtypes
    sel = np.zeros((2 * C, C * 128), dtype=ml_dtypes.bfloat16)
    for c in range(C):
        sel[c, c * 128:(c + 1) * 128] = 1.0
        sel[C + c, c * 128:(c + 1) * 128] = 1.0
    ident = np.eye(128, dtype=np.float32)

    in_maps = []
    for k in range(NCORES):
        bb, half = divmod(k, 2)
        i0 = half * ROWS
        in_maps.append({
            "hT": np.ascontiguousarray(h[bb].T),
            "hTi": np.ascontiguousarray(h[bb, i0:i0 + ROWS].T),
            "WT": WT,
            "bgb": bgb,
            "sel": sel,
            "identity": ident,
        })
    return in_maps


def run(h, W, b, gamma, beta, trace=False, **trace_kwargs):
    if "nc" not in _CACHE:
        _CACHE["nc"] = _build_program()
    nc = _CACHE["nc"]
    res = run_bass_kernel_spmd(
        nc,
        _in_maps(h, W, b, gamma, beta),
        core_ids=list(range(NCORES)),
        trace=trace,
        **trace_kwargs,
    )
    out = np.zeros((B, N, N, C), dtype=np.float32)
    for k in range(NCORES):
        bb, half = divmod(k, 2)
        i0 = half * ROWS
        out[bb, i0:i0 + ROWS] = res.results[k]["out"].reshape(ROWS, N, C)
    return out, res


def kernel(h, W, b, gamma, beta):
    out, _ = run(h, W, b, gamma, beta)
    return out


# revision 24
# speedup vs baseline: 1.8085x; 1.0059x over previous
"""Trainium2 Bass kernel for nn_DistanceModule.

Computes, for h [4,512,64], W [64,64], b/gamma/beta [64]:
    x = LayerNorm(ReLU(h @ W.T + b))          # [B,N,C]
    D[b,i,j,c] = x[b,i,c] * x[b,j,c]
    out = softmax(D, axis=-1)                 # [B,N,N,C] f32 (256 MB)

Sharding: 2048 (b,i) rows split across 8 cores -> 256 rows/core
(core k: batch b=k//2, i in [256*(k%2), 256*(k%2)+256)). Each core
computes x[b] on-chip, then streams its [256, 512, 64] output slice.

Per-core pipeline (all engines overlapped, per (i-tile, j-block) chunk):
  PE    : selector-matmul broadcasts xT row c across 128 partitions (PSUM)
  ScalarE: exp(bcast_c * x_i[:,c]) fused multiply+exp, per c
  VectorE: segmented reduce_sum over c, reciprocal, normalize multiply
  DMA   : 8 MB contiguous store per chunk

Softmax is computed without max-subtraction: LayerNorm bounds |x| by
sqrt(C-1) ~= 7.94, so logits <= 63 and exp <= 2.4e27 < f32 max.
"""

import numpy as np

import concourse.bacc as bacc
import concourse.bass as bass
import concourse.mybir as mybir
import concourse.tile as tile
from concourse.bass_utils import run_bass_kernel_spmd

B, N, C = 4, 512, 64
NCORES = 8
ROWS = 256          # (b,i) rows per core
JBLK = 256          # j-block width
EPS = 1e-5
F32 = mybir.dt.float32
BF16 = mybir.dt.bfloat16

_CACHE = {}


def _build_program():
    nc = bacc.Bacc(
        "TRN2",
        target_bir_lowering=False,
        debug=False,
        enable_asserts=False,
        num_devices=NCORES,
    )

    hT_d = nc.dram_tensor("hT", [C, N], F32, kind="ExternalInput")
    hTi_d = nc.dram_tensor("hTi", [C, ROWS], F32, kind="ExternalInput")
    WT_d = nc.dram_tensor("WT", [C, C], F32, kind="ExternalInput")
    bgb_d = nc.dram_tensor("bgb", [128, 3 * C], F32, kind="ExternalInput")
    sel_d = nc.dram_tensor("sel", [2 * C, C * 128], BF16, kind="ExternalInput")
    id_d = nc.dram_tensor("identity", [128, 128], F32, kind="ExternalInput")
    out_d = nc.dram_tensor("out", [ROWS, N * C], F32, kind="ExternalOutput")

    X = mybir.AxisListType.X
    sub = mybir.AluOpType.subtract
    mult = mybir.AluOpType.mult
    Exp = mybir.ActivationFunctionType.Exp
    Sqrt = mybir.ActivationFunctionType.Sqrt

    with tile.TileContext(nc) as tc:
        with tc.tile_pool(name="const", bufs=1) as constp:
            hTi = constp.tile([C, ROWS], F32)
            nc.sync.dma_start(hTi[:], hTi_d[:])
            hT = constp.tile([C, N], F32)
            nc.sync.dma_start(hT[:], hT_d[:])
            WT = constp.tile([C, C], F32)
            nc.sync.dma_start(WT[:], WT_d[:])
            bgb = constp.tile([128, 3 * C], F32)
            nc.sync.dma_start(bgb[:], bgb_d[:])
            sel = constp.tile([2 * C, C * 128], BF16)
            nc.sync.dma_start(sel[:], sel_d[:])
            ident = constp.tile([128, 128], F32)
            nc.sync.dma_start(ident[:], id_d[:])

            xT = constp.tile([C, N], F32)          # x[b].T  (c on partitions)
            xi = constp.tile([128, 2, C], F32)     # this core's two i-tiles
            xT_hilo = constp.tile([128, N], BF16)  # K-stacked bf16 hi/lo of xT
            eps_t = constp.tile([128, 1], F32)
            nc.vector.memset(eps_t[:], EPS)

            # ---- x = LayerNorm(ReLU(h @ W.T + b)) --------------------------
            # i-tiles (t=4,5) first so the main loop's scale operand is ready
            # early; each xT slice gets its bf16 hi/lo split as soon as it is
            # transposed, letting the first broadcast matmuls start before
            # the whole prep finishes.
            with (
                tc.tile_pool(name="xprep", bufs=2) as xprep,
                tc.tile_pool(name="psum_prep", bufs=2, space=bass.MemorySpace.PSUM) as psp,
                tc.tile_pool(name="psum_tp", bufs=2, space=bass.MemorySpace.PSUM) as ptp,
            ):
                for t in (4, 5, 0, 1, 2, 3):
                    if t < 4:
                        lhsT = hT[:, t * 128:(t + 1) * 128]
                    else:
                        lhsT = hTi[:, (t - 4) * 128:(t - 3) * 128]
                    xp = psp.tile([128, C], F32, tag="xp")
                    nc.tensor.matmul(xp[:], lhsT, WT[:])
                    xs = xprep.tile([128, C], F32, tag="xs")
                    nc.vector.tensor_add(xs[:], xp[:], bgb[:, 0:C])       # + b
                    nc.scalar.activation(
                        xs[:], xs[:], mybir.ActivationFunctionType.Relu
                    )
                    stats = xprep.tile([128, 6], F32, tag="stats")
                    nc.vector.bn_stats(stats[:], xs[:])
                    mv = xprep.tile([128, 2], F32, tag="mv")
                    nc.vector.bn_aggr(mv[:], stats[:])
                    std = xprep.tile([128, 1], F32, tag="std")
                    nc.scalar.activation(std[:], mv[:, 1:2], Sqrt, bias=eps_t[:, 0:1])
                    rstd = xprep.tile([128, 1], F32, tag="rstd")
                    nc.vector.reciprocal(rstd[:], std[:])
                    xn = xprep.tile([128, C], F32, tag="xn")
                    nc.vector.tensor_scalar(
                        xn[:], xs[:], mv[:, 0:1], rstd[:, 0:1], op0=sub, op1=mult
                    )
                    nc.vector.tensor_mul(xn[:], xn[:], bgb[:, C:2 * C])   # * gamma
                    nc.vector.tensor_add(xn[:], xn[:], bgb[:, 2 * C:3 * C])  # + beta
                    if t < 4:
                        tp = ptp.tile([C, 128], F32, tag="tp")
                        nc.tensor.transpose(tp[:], xn[:], ident[:])
                        sl = slice(t * 128, (t + 1) * 128)
                        nc.vector.tensor_copy(xT[:, sl], tp[:])
                        # K-stacked bf16 hi/lo split of this slice:
                        # partitions 0-63 hold bf16(x), 64-127 bf16(x - hi).
                        # One K=128 matmul with the doubled selector then
                        # sums both rank-64 halves in PSUM fp32, reproducing
                        # the f32 broadcast exactly to ~2^-17 in one pass.
                        nc.vector.tensor_copy(xT_hilo[0:C, sl], xT[:, sl])
                        hi32 = xprep.tile([C, 128], F32, tag="hi32")
                        nc.vector.tensor_copy(hi32[:], xT_hilo[0:C, sl])
                        nc.vector.tensor_sub(xT_hilo[C:2 * C, sl], xT[:, sl], hi32[:])
                    else:
                        nc.vector.tensor_copy(xi[:, t - 4, :], xn[:])

            # ---- main: exp(x_i * x_j), softmax over c, store ---------------
            # Chunk widths are asymmetric: a narrow first chunk lets the
            # vector engine (the critical path) start early, and a narrow
            # last chunk shrinks the drain tail. Still 2 activation
            # instructions per (i-tile, c), so ScalarE time is unchanged.
            CHUNKS = {0: (224, 288), 1: (288, 224)}
            with (
                tc.tile_pool(name="main", bufs=2) as mainp,
                tc.tile_pool(name="small", bufs=3) as smallp,
                tc.tile_pool(name="psum_bc", bufs=6, space=bass.MemorySpace.PSUM) as pbc,
            ):
                for it in range(2):
                    j0 = 0
                    for jw in CHUNKS[it]:
                        expt = mainp.tile([128, jw, C], F32, tag="exp")
                        for c in range(C):
                            bc = pbc.tile([128, jw], F32, tag="bc")
                            nc.tensor.matmul(
                                bc[:],
                                sel[:, c * 128:(c + 1) * 128],
                                xT_hilo[:, j0:j0 + jw],
                            )
                            nc.scalar.activation(
                                expt[:, :, c], bc[:], Exp, scale=xi[:, it, c:c + 1]
                            )
                        sums = smallp.tile([128, jw], F32, tag="sums")
                        nc.vector.reduce_sum(sums[:], expt[:], axis=X)
                        recip = smallp.tile([128, jw], F32, tag="recip")
                        nc.vector.reciprocal(recip[:], sums[:])
                        # normalize in j-quarters; each quarter DMAs out as
                        # soon as it is scaled (frees the exp buffer sooner
                        # and overlaps store with compute).
                        QW = jw // 4
                        for q in range(4):
                            sl = slice(q * QW, (q + 1) * QW)
                            nc.vector.tensor_mul(
                                expt[:, sl, :],
                                expt[:, sl, :],
                                recip[:, sl][:, :, None].broadcast_to((128, QW, C)),
                            )
                            nc.sync.dma_start(
                                out_d[it * 128:(it + 1) * 128,
                                      (j0 + q * QW) * C:(j0 + (q + 1) * QW) * C],
                                expt[:, sl, :].rearrange("p j c -> p (j c)"),
                            )
                        j0 += jw
    nc.compile()
    return nc


def _in_maps(h, W, b, gamma, beta):
    h = np.asarray(h, dtype=np.float32)
    W = np.asarray(W, dtype=np.float32)
    b = np.asarray(b, dtype=np.float32)
    gamma = np.asarray(gamma, dtype=np.float32)
    beta = np.asarray(beta, dtype=np.float32)

    WT = np.ascontiguousarray(W.T)
    bgb = np.ascontiguousarray(
        np.broadcast_to(np.concatenate([b, gamma, beta])[None, :], (128, 3 * C))
    )
    import ml_dtypes
    sel = np.zeros((2 * C, C * 128), dtype=ml_dtypes.bfloat16)
    for c in range(C):
        sel[c, c * 128:(c + 1) * 128] = 1.0
        sel[C + c, c * 128:(c + 1) * 128] = 1.0
    ident = np.eye(128, dtype=np.float32)

    in_maps = []
    for k in range(NCORES):
        bb, half = divmod(k, 2)
        i0 = half * ROWS
        in_maps.append({
            "hT": np.ascontiguousarray(h[bb].T),
            "hTi": np.ascontiguousarray(h[bb, i0:i0 + ROWS].T),
            "WT": WT,
            "bgb": bgb,
            "sel": sel,
            "identity": ident,
        })
    return in_maps


def run(h, W, b, gamma, beta, trace=False, **trace_kwargs):
    if "nc" not in _CACHE:
        _CACHE["nc"] = _build_program()
    nc = _CACHE["nc"]
    res = run_bass_kernel_spmd(
        nc,
        _in_maps(h, W, b, gamma, beta),
        core_ids=list(range(NCORES)),
        trace=trace,
        **trace_kwargs,
    )
    out = np.zeros((B, N, N, C), dtype=np.float32)
    for k in range(NCORES):
        bb, half = divmod(k, 2)
        i0 = half * ROWS
        out[bb, i0:i0 + ROWS] = res.results[k]["out"].reshape(ROWS, N, C)
    return out, res


def kernel(h, W, b, gamma, beta):
    out, _ = run(h, W, b, gamma, beta)
    return out
